# revision 23
# baseline (speedup 1.0000x reference)
"""Trainium2 Bass kernel for nn_CrossAttentionBlock (B=4, T=4096, C=512, H=8,
INNER=2048, NIN=2) on 8 NeuronCores.

Sharding: core c handles batch b=c//2, token half h=c%2 (2048 tokens each).
Cross-core coupling: only the linear-attention context (ctx = k^T v) and
k_sum, pair-wise AllReduced (cores 2b, 2b+1).

v2 design notes (vs the v1 feature-major baseline):
- LN gains folded into weights host-side; LN bias folded into projection
  biases. The per-token mean-shift enters each projection through a K<=2
  "seed" matmul (beta (x) u + 1 (x) b') that replaces the plain bias matmul.
- inv_std = exp(-0.5*ln(var+eps)) so LN, softmax-exp and all copies share
  ONE scalar-engine activation table (natural_log_exp family); gelu is the
  only other table -> ~4 table loads total.
- All heavy matmuls in bf16 (1 cyc/row incl. the 65-row ctx matmuls that
  were 4 cyc/row in fp32r); LN stats matmuls stay fp32r; residual fp32.
- scalar_tensor_tensor reads broadcast/projection results straight from
  PSUM (no PSUM->SBUF copy ops); squares and k-normalization run on the
  idle gpsimd (Pool) engine; reciprocals use reciprocal_approx_fast.
- attn combine: out = E/S + sum_i (E@ctx_i)/G_i is computed as
  qn + sum_i BD_i.T @ (E o bc(1/G_i)) with the per-head scale applied to E
  BEFORE the block-diag matmul (legal: scale is constant within a head),
  so the BD products accumulate in PSUM.
- ks_i emerges as an extra all-ones column in the ctx matmul, pre-laid-out
  in the [128, 260] folded DRAM tile that the AllReduce moves, so SG/BD
  assembly after the collective is a handful of plain DMAs.
- Phase order: kv/ctx first, then the collective overlaps the q/E (and E2)
  production.
"""
import os
import numpy as np

import concourse.bass as bass
import concourse.tile as tile
from concourse import mybir
from concourse.vector_clock import ScopedClock
from concourse.bass_utils import run_bass_kernel_spmd

F32 = mybir.dt.float32
F32R = mybir.dt.float32r
BF16 = mybir.dt.bfloat16
AF = mybir.ActivationFunctionType
OP = mybir.AluOpType

B, T, C, H, D, INNER, NIN = 4, 4096, 512, 8, 64, 2048, 2
N_CORES = 8
NTOK = 2048          # tokens per core
CHUNK = 512          # tokens per chunk
NCH = NTOK // CHUNK  # 4 chunks
FT = C // 128        # 4 feature tiles
IT = INNER // 128    # 16 inner tiles
HB = 65              # head block width in ctx psum (64 v cols + 1 ks col)
LN_EPS = 1e-5
GROUPS = [[0, 1], [2, 3], [4, 5], [6, 7]]

_split_counter = [0]


def _split_multi_waits(nc):
    """This walrus build only supports one sync-wait per instruction; move
    extra waits onto same-engine NoOps placed immediately before."""
    for f in nc.m.functions:
        for blk in f.blocks:
            out = []
            changed = False
            for inst in blk.instructions:
                si = inst.sync_info
                if si is not None and si.on_wait and len(si.on_wait) > 1:
                    waits = list(si.on_wait)
                    for w in waits[:-1]:
                        _split_counter[0] += 1
                        nop = mybir.InstNoOp(
                            name=f"I-waitsplit-{_split_counter[0]}", ins=[], outs=[]
                        )
                        nop.engine = inst.engine
                        nop.sync_info = mybir.SyncInfo(on_wait=[w], on_update=[])
                        out.append(nop)
                    si.on_wait = waits[-1:]
                    inst.sync_info = si
                    changed = True
                out.append(inst)
            if changed:
                blk.instructions = out


class _TC(tile.TileContext):
    def _drain_and_barrier(self, tick_clock, wait_clock):
        drain_inst = self.nc.sync.drain()
        wait_clock.add_sem_waits(
            drain_inst.ins, ScopedClock({None: tick_clock.global_clock})
        )
        si = drain_inst.ins.sync_info
        if si is not None and si.on_wait and len(si.on_wait) > 1:
            waits = list(si.on_wait)
            si.on_wait = waits[:1]
            drain_inst.ins.sync_info = si
            for i in range(1, len(waits)):
                extra = self.nc.sync.drain()
                esi = extra.ins.sync_info
                if esi is None:
                    extra.ins.sync_info = mybir.SyncInfo(
                        on_wait=waits[i : i + 1], on_update=[]
                    )
                else:
                    esi.on_wait = waits[i : i + 1]
                    extra.ins.sync_info = esi
        self.nc.all_engine_barrier()
        assert self.sems is not None
        popped = self.nc._tile_sem_poison_stack.pop()
        assert popped is self._sem_poison
        self.nc.clear_and_free_semaphores(list(self.sems.allocated().values()))
        self.nc.all_engine_barrier()


def _build_program(split=True):
    nc = bass.Bass("TRN2", target_bir_lowering=False, debug=False, num_devices=N_CORES)
    I = {}

    def di(name, shape, dt=F32):
        I[name] = nc.dram_tensor(name, list(shape), dt, kind="ExternalInput").ap()

    di("xT", [C, NTOK])
    di("ysT", [NIN, C, NTOK])
    # bf16 weights, [in, out] layout, LN gains folded where applicable
    di("wq", [C, C], BF16)
    di("wo", [C, C], BF16)
    di("saq", [C, C], BF16)
    di("sao", [C, C], BF16)
    di("sak", [C, C], BF16)
    di("sav", [C, C], BF16)
    di("wk", [NIN, C, C], BF16)
    di("wv", [NIN, C, C], BF16)
    di("f1w1", [C, INNER], BF16)
    di("f1w2", [INNER, C], BF16)
    di("f2w1", [C, INNER], BF16)
    di("f2w2", [INNER, C], BF16)
    # seeds: FM u-rows [FT, 1, 128]; TM [2, C] = [u; b'] stacks
    di("fmsq", [FT, 1, 128], BF16)
    di("fmssaq", [FT, 1, 128], BF16)
    di("kvsd", [NIN, 2, 2, C], BF16)   # [i][k/v] -> [u; b'] rows
    di("sasd", [2, 2, C], BF16)        # [k/v] -> [u; b'] rows
    # bias cols fp32 (per-partition activation biases / stt scalars)
    di("bq_c", [128, FT])
    di("bsaq_c", [128, FT])
    di("bo_c", [128, FT])
    di("bsao_c", [128, FT])
    di("f1b1_c", [128, IT])
    di("f2b1_c", [128, IT])
    di("f1b2_c", [128, FT])
    di("f2b2_c", [128, FT])
    di("ones_c", [1, 128])
    di("ones_r", [128, 1])
    di("sgbase", [FT, 128, 24], BF16)
    di("sel8", [FT, 8, 128], BF16)

    out_t = nc.dram_tensor("outT", [C, NTOK], F32, kind="ExternalOutput").ap()

    with _TC(nc) as tc:
        _Emitter(nc, tc, I, out_t).run()
    if split:
        _split_multi_waits(nc)
    from concourse.library_overlay import lower_extended_insts
    lower_extended_insts(nc)
    return nc


class _Emitter:
    def __init__(self, nc, tc, I, out_t):
        self.nc, self.tc, self.I, self.out_t = nc, tc, I, out_t

    # ---------------- layer norm front ----------------
    def ln_front(self, x_tiles, sq_engine="pool"):
        """Stats + rows for LN on fp32(r) feature-major tiles.
        Returns (A_ps [128,CHUNK] f32 PSUM broadcast of inv_std,
                 betaones [2,CHUNK] bf16 SBUF: row0=-m*inv_std, row1=1)."""
        nc = self.nc
        s_ps = self.p_stats.tile([1, CHUNK], F32, tag="stats", name="stats")
        for k in range(FT):
            nc.tensor.matmul(s_ps, self.ONESR, x_tiles[k],
                             start=(k == 0), stop=(k == FT - 1))
        xsq = []
        for k in range(FT):
            sq = self.sqp.tile([128, CHUNK], F32R, tag="xsq", name="xsq")
            if sq_engine == "pool":
                with nc.allow_low_precision(reason="fp32r feeds matmul"):
                    nc.gpsimd.tensor_tensor(out=sq, in0=x_tiles[k].bitcast(F32),
                                            in1=x_tiles[k].bitcast(F32),
                                            op=OP.mult)
            else:
                nc.scalar.activation(out=sq, in_=x_tiles[k].bitcast(F32),
                                     func=AF.Square)
            xsq.append(sq)
        q_ps = self.p_stats.tile([1, CHUNK], F32, tag="stats", name="stats")
        for k in range(FT):
            nc.tensor.matmul(q_ps, self.ONESR, xsq[k],
                             start=(k == 0), stop=(k == FT - 1))
        # rows: mrow = -mean; var = E[x^2] - mean^2; alpha = rsqrt(var+eps)
        mrow = self.rows.tile([1, CHUNK], F32, tag="rows", name="rows")
        nc.vector.tensor_scalar(out=mrow, in0=s_ps, scalar1=-1.0 / C,
                                scalar2=None, op0=OP.mult)
        m2 = self.rows.tile([1, CHUNK], F32, tag="rows", name="rows")
        nc.vector.tensor_tensor(out=m2, in0=mrow, in1=mrow, op=OP.mult)
        var = self.rows.tile([1, CHUNK], F32, tag="rows", name="rows")
        nc.vector.scalar_tensor_tensor(out=var, in0=q_ps, scalar=1.0 / C,
                                       in1=m2, op0=OP.mult, op1=OP.subtract)
        lnv = self.rows.tile([1, CHUNK], F32, tag="rows", name="rows")
        nc.scalar.activation(out=lnv, in_=var, func=AF.Ln, bias=self.EPS,
                             scale=1.0)
        alpha = self.rows.tile([1, CHUNK], F32R, tag="rows", name="rows")
        with nc.allow_low_precision(reason="fp32r feeds matmul"):
            nc.scalar.activation(out=alpha, in_=lnv, func=AF.Exp,
                                 bias=self.ZERO1, scale=-0.5)
        bo = self.bop.tile([2, CHUNK], BF16, tag="bo", name="bo")
        nc.vector.memset(bo, 1.0)
        with nc.allow_low_precision(reason="seed row"):
            nc.vector.tensor_tensor(out=bo[0:1, :], in0=mrow,
                                    in1=alpha.bitcast(F32), op=OP.mult)
        a_ps = self.p_bc.tile([128, CHUNK], F32, tag="bc", name="bc")
        nc.tensor.matmul(a_ps, self.ONESC, alpha, start=True, stop=True)
        return a_ps, bo

    def ln_apply(self, x_tiles, a_ps, pool, tag):
        """xn[k] = x[k] * bc(inv_std)  (bf16, mean-shift via seed matmuls)"""
        nc = self.nc
        outs = []
        for k in range(FT):
            xk = pool.tile([128, CHUNK], BF16, tag=tag, name=tag)
            nc.vector.scalar_tensor_tensor(
                out=xk, in0=x_tiles[k].bitcast(F32), scalar=1.0, in1=a_ps,
                op0=OP.mult, op1=OP.mult)
            outs.append(xk)
        return outs

    def load_w512(self, ap, pool, tag, width=C):
        tiles = []
        for k in range(FT):
            t = pool.tile([128, width], BF16, tag=f"{tag}{k}", name=f"{tag}{k}")
            self.nc.sync.dma_start(out=t, in_=ap[128 * k : 128 * (k + 1), :])
            tiles.append(t)
        return tiles

    # ---------------- kv + ctx pipeline (token-major) ----------------
    def kv_ctx(self, zy, bo, WK, WV, sdk, sdv, ctx_ps, first, last):
        """One (chunk, input) step: k/v proj + softmax-k + ctx accumulation.
        zy: 4 bf16 FM tiles; bo: [2,CHUNK] betaones; sdk/sdv: [2,C] moving
        seed rows; ctx_ps: [128, 4*HB] psum tile (even heads rows 0:64,
        odd heads rows 64:128)."""
        nc = self.nc
        for t in range(FT):
            kps = self.p_kv.tile([128, C], F32, tag="kv", name="kv")
            nc.tensor.matmul(kps, bo[:, 128 * t : 128 * (t + 1)], sdk,
                             start=True, stop=False)
            for k in range(FT):
                nc.tensor.matmul(kps, zy[k][:, 128 * t : 128 * (t + 1)],
                                 WK[k], start=False, stop=(k == FT - 1))
            kE = self.kep.tile([128, C], BF16, tag="kE", name="kE")
            nc.scalar.activation(out=kE, in_=kps, func=AF.Exp,
                                 bias=self.ZERO128, scale=1.0)
            ssum = self.smallp.tile([128, H], F32, tag="ssum", name="ssum")
            nc.vector.tensor_reduce(
                out=ssum, in_=kE.rearrange("p (h d) -> p h d", d=D),
                axis=mybir.AxisListType.X, op=OP.add)
            rsum = self.smallp.tile([128, H], F32, tag="rsum", name="rsum")
            nc.vector.reciprocal(out=rsum, in_=ssum)
            kn = self.knp.tile([128, C], BF16, tag="kn", name="kn")
            for h in range(H):
                nc.vector.tensor_scalar(
                    out=kn[:, D * h : D * (h + 1)],
                    in0=kE[:, D * h : D * (h + 1)],
                    scalar1=rsum[:, h : h + 1], scalar2=None, op0=OP.mult)
            vps = self.p_kv.tile([128, C], F32, tag="kv", name="kv")
            nc.tensor.matmul(vps, bo[:, 128 * t : 128 * (t + 1)], sdv,
                             start=True, stop=False)
            for k in range(FT):
                nc.tensor.matmul(vps, zy[k][:, 128 * t : 128 * (t + 1)],
                                 WV[k], start=False, stop=(k == FT - 1))
            va = self.vap.tile([128, H * HB], BF16, tag="va", name="va")
            nc.scalar.activation(
                out=va.rearrange("p (h b) -> p h b", b=HB)[:, :, 0:D],
                in_=vps.rearrange("p (h d) -> p h d", d=D),
                func=AF.Copy, bias=0.0, scale=1.0)
            nc.vector.memset(
                va.rearrange("p (h b) -> p h b", b=HB)[:, :, D : D + 1], 1.0)
            st = first and t == 0
            sp = last and t == FT - 1
            for h in range(H):
                half, c = h % 2, h // 2
                nc.tensor.matmul(
                    ctx_ps[64 * half : 64 * half + 64, HB * c : HB * (c + 1)],
                    kn[:, D * h : D * (h + 1)],
                    va[:, HB * h : HB * (h + 1)],
                    start=st, stop=sp,
                    tile_position=(0, 64 * half))

    # ---------------- attention back ----------------
    def attn_back(self, X, E, cc_out, n_in, wo_ap, bo_cols, tagp, Xnew_pool):
        """out = E/S + sum_i BD_i.T @ (E o bc(1/G_i)); then wo proj+residual."""
        nc, tc, I = self.nc, self.tc, self.I
        Xout = [[None] * FT for _ in range(NCH)]
        cc = (lambda i: cc_out[i]) if n_in > 1 else (lambda i: cc_out)
        ncols = 8 + 8 * n_in
        with tc.tile_pool(name=f"w_{tagp}", bufs=1) as w_o, \
             tc.tile_pool(name=f"as_{tagp}", bufs=1) as attn_s, \
             tc.tile_pool(name=f"ao_{tagp}", bufs=10) as aop, \
             tc.tile_pool(name=f"rec_{tagp}", bufs=4) as recp, \
             tc.tile_pool(name=f"pg_{tagp}", bufs=1, space="PSUM") as p_g, \
             tc.tile_pool(name=f"pr_{tagp}", bufs=3, space="PSUM") as p_r:
            WO = self.load_w512(wo_ap, w_o, "wo")
            # SG tiles: base pattern + ks columns from cc_out, then -> bf16
            SGT = []
            for c in range(FT):
                sg = attn_s.tile([128, ncols], BF16, tag=f"sg{c}", name=f"sg{c}")
                nc.sync.dma_start(out=sg, in_=I["sgbase"][c][:, 0:ncols])
                for i in range(n_in):
                    col = 8 + 8 * i + 2 * c
                    nc.gpsimd.dma_start(
                        out=sg[0:64, col : col + 1],
                        in_=cc(i)[0:64, HB * c + D : HB * c + D + 1])
                    nc.gpsimd.dma_start(
                        out=sg[64:128, col + 1 : col + 2],
                        in_=cc(i)[64:128, HB * c + D : HB * c + D + 1])
                SGT.append(sg)
            BD = [[None] * FT for _ in range(n_in)]
            for i in range(n_in):
                for c in range(FT):
                    bd = attn_s.tile([128, 128], BF16, tag=f"bd{i}_{c}",
                                     name=f"bd{i}_{c}")
                    nc.vector.memset(bd, 0.0)
                    nc.gpsimd.dma_start(
                        out=bd[0:64, 0:64],
                        in_=cc(i)[0:64, HB * c : HB * c + D])
                    nc.gpsimd.dma_start(
                        out=bd[64:128, 64:128],
                        in_=cc(i)[64:128, HB * c : HB * c + D])
                    BD[i][c] = bd

            for ch in range(NCH):
                recs = []
                for j in range(1 + n_in):
                    gps = p_g.tile([8, CHUNK], F32, tag="gps", name="gps")
                    for c in range(FT):
                        nc.tensor.matmul(gps, SGT[c][:, 8 * j : 8 * (j + 1)],
                                         E[ch][c], start=(c == 0),
                                         stop=(c == FT - 1))
                    r = recp.tile([8, CHUNK], F32, tag="rec", name="rec")
                    nc.vector.reciprocal_approx_fast(out=r, in_=gps)
                    rb = recp.tile([8, CHUNK], BF16, tag="recb", name="recb")
                    nc.vector.tensor_scalar(out=rb, in0=r, scalar1=1.0,
                                            scalar2=None, op0=OP.mult)
                    recs.append(rb)
                outc = []
                for c in range(FT):
                    Rps = []
                    for j in range(1 + n_in):
                        rp = p_r.tile([128, CHUNK], F32, tag="R", name="R")
                        nc.tensor.matmul(rp, self.SEL8[c], recs[j],
                                         start=True, stop=True)
                        Rps.append(rp)
                    qn = aop.tile([128, CHUNK], BF16, tag="qn", name="qn")
                    nc.vector.scalar_tensor_tensor(
                        out=qn, in0=E[ch][c], scalar=1.0, in1=Rps[0],
                        op0=OP.mult, op1=OP.mult)
                    bd_ps = self.p_mm.tile([128, CHUNK], F32, tag="mm", name="mm")
                    for i in range(n_in):
                        qh = aop.tile([128, CHUNK], BF16, tag="qh", name="qh")
                        nc.vector.scalar_tensor_tensor(
                            out=qh, in0=E[ch][c], scalar=1.0, in1=Rps[1 + i],
                            op0=OP.mult, op1=OP.mult)
                        nc.tensor.matmul(bd_ps, BD[i][c], qh,
                                         start=(i == 0), stop=(i == n_in - 1))
                    ao = aop.tile([128, CHUNK], BF16, tag="ao", name="ao")
                    nc.vector.scalar_tensor_tensor(
                        out=ao, in0=qn, scalar=0.0, in1=bd_ps,
                        op0=OP.add, op1=OP.add)
                    outc.append(ao)
                for m in range(FT):
                    wps = self.p_mm.tile([128, CHUNK], F32, tag="mm", name="mm")
                    for c in range(FT):
                        nc.tensor.matmul(wps, WO[c][:, 128 * m : 128 * (m + 1)],
                                         outc[c], start=(c == 0),
                                         stop=(c == FT - 1))
                    xo = Xnew_pool.tile([128, CHUNK], F32R, tag="resid",
                                        name="resid")
                    with nc.allow_low_precision(reason="fp32r resid"):
                        nc.vector.scalar_tensor_tensor(
                            out=xo, in0=X[ch][m].bitcast(F32),
                            scalar=bo_cols[:, m : m + 1], in1=wps,
                            op0=OP.add, op1=OP.add)
                    Xout[ch][m] = xo
        return Xout

    # ---------------- FFN ----------------
    def ffn(self, Xin, w1name, w2name, B1, B2):
        nc, tc, I = self.nc, self.tc, self.I
        Xout = [[None] * FT for _ in range(NCH)]
        with tc.tile_pool(name=w1name, bufs=1) as w1p, \
             tc.tile_pool(name=w2name + "s", bufs=8) as w2p, \
             tc.tile_pool(name=w1name + "h", bufs=4) as hp, \
             tc.tile_pool(name=w1name + "x", bufs=10) as xnp, \
             tc.tile_pool(name=w1name + "b", bufs=2) as bbp, \
             tc.tile_pool(name=w1name + "p", bufs=4, space="PSUM") as p_ffn:
            W1 = []
            for k in range(FT):
                t = w1p.tile([128, INNER], BF16, tag=f"w1_{k}", name=f"w1_{k}")
                nc.sync.dma_start(
                    out=t, in_=I[w1name][128 * k : 128 * (k + 1), :])
                W1.append(t)
            for ch in range(NCH):
                a_ps, bo = self.ln_front(Xin[ch])
                b_ps = self.p_bc.tile([128, CHUNK], F32, tag="bc", name="bc")
                nc.tensor.matmul(b_ps, self.ONESCB, bo[0:1, :], start=True,
                                 stop=True)
                bsb = bbp.tile([128, CHUNK], F32, tag="bsb", name="bsb")
                nc.scalar.activation(out=bsb, in_=b_ps, func=AF.Copy,
                                     bias=0.0, scale=1.0)
                xn = []
                for k in range(FT):
                    u = xnp.tile([128, CHUNK], F32, tag="u", name="u")
                    nc.vector.scalar_tensor_tensor(
                        out=u, in0=Xin[ch][k].bitcast(F32), scalar=1.0,
                        in1=a_ps, op0=OP.mult, op1=OP.mult)
                    xk = xnp.tile([128, CHUNK], BF16, tag="xn", name="xn")
                    nc.vector.tensor_tensor(out=xk, in0=u, in1=bsb, op=OP.add)
                    xn.append(xk)
                ops = [p_ffn.tile([128, CHUNK], F32, tag="ffn", name="ffn")
                       for _ in range(FT)]
                for k in range(IT):
                    hps = self.p_mm.tile([128, CHUNK], F32, tag="mm", name="mm")
                    for c in range(FT):
                        nc.tensor.matmul(hps, W1[c][:, 128 * k : 128 * (k + 1)],
                                         xn[c], start=(c == 0),
                                         stop=(c == FT - 1))
                    h = hp.tile([128, CHUNK], BF16, tag="h", name="h")
                    nc.scalar.activation(out=h, in_=hps, func=AF.Gelu_apprx_tanh,
                                         bias=B1[:, k : k + 1], scale=1.0)
                    w2t = w2p.tile([128, C], BF16, tag="w2s", name="w2s")
                    nc.sync.dma_start(
                        out=w2t, in_=I[w2name][128 * k : 128 * (k + 1), :])
                    for m in range(FT):
                        nc.tensor.matmul(ops[m],
                                         w2t[:, 128 * m : 128 * (m + 1)], h,
                                         start=(k == 0), stop=(k == IT - 1))
                for m in range(FT):
                    xo = self.resid.tile([128, CHUNK], F32R, tag="resid",
                                         name="resid")
                    with nc.allow_low_precision(reason="fp32r resid"):
                        nc.vector.scalar_tensor_tensor(
                            out=xo, in0=Xin[ch][m].bitcast(F32),
                            scalar=B2[:, m : m + 1], in1=ops[m],
                            op0=OP.add, op1=OP.add)
                    Xout[ch][m] = xo
        return Xout

    # ---------------- q / E production (feature-major) ----------------
    def q_exp(self, Xin, wname, fmname, bcol, lnpack, Epool, sq_engine="pool"):
        """E[ch][m] = exp(Wq_g @ (x o bc(alpha)) + u (x) beta + b') for all
        chunks. lnpack: None (LN computed here per chunk) or a list of
        (bo, xn_tiles) per chunk."""
        nc, tc, I = self.nc, self.tc, self.I
        E = [[None] * FT for _ in range(NCH)]
        with tc.tile_pool(name=f"w_{wname}", bufs=1) as w_q, \
             tc.tile_pool(name=f"fms_{wname}", bufs=1) as fmsp, \
             tc.tile_pool(name=f"xn_{wname}", bufs=18) as xnp:
            WQ = self.load_w512(I[wname], w_q, "wq")
            FMS = []
            for m in range(FT):
                s = fmsp.tile([1, 128], BF16, tag=f"fms{m}", name=f"fms{m}")
                nc.sync.dma_start(out=s, in_=I[fmname][m])
                FMS.append(s)
            for ch in range(NCH):
                if lnpack is None:
                    a_ps, bo = self.ln_front(Xin[ch], sq_engine=sq_engine)
                    xn = self.ln_apply(Xin[ch], a_ps, xnp, "xn")
                else:
                    bo, xn = lnpack[ch]
                for m in range(FT):
                    ps = self.p_mm.tile([128, CHUNK], F32, tag="mm", name="mm")
                    nc.tensor.matmul(ps, FMS[m], bo[0:1, :], start=True,
                                     stop=False)
                    for k in range(FT):
                        nc.tensor.matmul(ps, WQ[k][:, 128 * m : 128 * (m + 1)],
                                         xn[k], start=False,
                                         stop=(k == FT - 1))
                    e = Epool.tile([128, CHUNK], BF16, tag="E", name="E")
                    nc.scalar.activation(out=e, in_=ps, func=AF.Exp,
                                         bias=bcol[:, m : m + 1], scale=1.0)
                    E[ch][m] = e
        return E

    # ---------------- main ----------------
    def run(self):
        nc, tc, I = self.nc, self.tc, self.I
        from contextlib import ExitStack

        with ExitStack() as ctx:
            const = ctx.enter_context(tc.tile_pool(name="const", bufs=1))
            self.resid = ctx.enter_context(tc.tile_pool(name="resid", bufs=20))
            self.epool = ctx.enter_context(tc.tile_pool(name="E", bufs=16))
            self.xn4p = ctx.enter_context(tc.tile_pool(name="xn4", bufs=16))
            self.rows = ctx.enter_context(tc.tile_pool(name="rows", bufs=10))
            self.bop = ctx.enter_context(tc.tile_pool(name="bop", bufs=10))
            self.sqp = ctx.enter_context(tc.tile_pool(name="sqp", bufs=5))
            dram = ctx.enter_context(tc.tile_pool(name="dram", bufs=1,
                                                  space="DRAM"))
            self.p_mm = ctx.enter_context(
                tc.tile_pool(name="p_mm", bufs=2, space="PSUM"))
            self.p_stats = ctx.enter_context(
                tc.tile_pool(name="p_stats", bufs=1, space="PSUM"))
            self.p_bc = ctx.enter_context(
                tc.tile_pool(name="p_bc", bufs=1, space="PSUM"))

            # ---------------- constants ----------------
            self.EPS = const.tile([1, 1], F32, tag="eps", name="eps")
            nc.vector.memset(self.EPS, LN_EPS)
            self.ZERO1 = const.tile([1, 1], F32, tag="z1", name="z1")
            nc.vector.memset(self.ZERO1, 0.0)
            self.ZERO128 = const.tile([128, 1], F32, tag="z128", name="z128")
            nc.vector.memset(self.ZERO128, 0.0)
            self.ONESC = const.tile([1, 128], F32R, tag="onesc", name="onesc")
            nc.sync.dma_start(out=self.ONESC, in_=I["ones_c"].bitcast(F32R))
            self.ONESCB = const.tile([1, 128], BF16, tag="onescb", name="onescb")
            nc.vector.memset(self.ONESCB, 1.0)
            self.ONESR = const.tile([128, 1], F32R, tag="onesr", name="onesr")
            nc.sync.dma_start(out=self.ONESR, in_=I["ones_r"].bitcast(F32R))
            self.SEL8 = []
            for c in range(FT):
                s = const.tile([8, 128], BF16, tag=f"sel8_{c}", name=f"sel8_{c}")
                nc.sync.dma_start(out=s, in_=I["sel8"][c])
                self.SEL8.append(s)

            def cols_tile(name, nt):
                t = const.tile([128, nt], F32, tag=name)
                nc.sync.dma_start(out=t, in_=I[name])
                return t

            BQ = cols_tile("bq_c", FT)
            BSAQ = cols_tile("bsaq_c", FT)
            BO = cols_tile("bo_c", FT)
            BSAO = cols_tile("bsao_c", FT)
            F1B1 = cols_tile("f1b1_c", IT)
            F1B2 = cols_tile("f1b2_c", FT)
            F2B1 = cols_tile("f2b1_c", IT)
            F2B2 = cols_tile("f2b2_c", FT)

            KVSD = []
            for i in range(NIN):
                sdk = const.tile([2, C], BF16, tag=f"sdk{i}", name=f"sdk{i}")
                nc.sync.dma_start(out=sdk, in_=I["kvsd"][i, 0])
                sdv = const.tile([2, C], BF16, tag=f"sdv{i}", name=f"sdv{i}")
                nc.sync.dma_start(out=sdv, in_=I["kvsd"][i, 1])
                KVSD.append((sdk, sdv))
            SASDK = const.tile([2, C], BF16, tag="sasdk", name="sasdk")
            nc.sync.dma_start(out=SASDK, in_=I["sasd"][0])
            SASDV = const.tile([2, C], BF16, tag="sasdv", name="sasdv")
            nc.sync.dma_start(out=SASDV, in_=I["sasd"][1])

            # ============ phase A: CA kv + ctx ============
            cc_in = dram.tile([NIN, 128, FT * HB], BF16, tag="cc_ca_in",
                              name="cc_ca_in")
            cc_out = dram.tile([NIN, 128, FT * HB], BF16, tag="cc_ca_out",
                               name="cc_ca_out")
            with tc.tile_pool(name="w_kv", bufs=1) as w_kv, \
                 tc.tile_pool(name="ysp", bufs=12) as ysp, \
                 tc.tile_pool(name="zyp", bufs=12) as zyp, \
                 tc.tile_pool(name="kep", bufs=2) as kep, \
                 tc.tile_pool(name="knp", bufs=2) as knp, \
                 tc.tile_pool(name="vap", bufs=2) as vap, \
                 tc.tile_pool(name="smallp", bufs=4) as smallp, \
                 tc.tile_pool(name="ctxsb", bufs=2) as ctxsb, \
                 tc.tile_pool(name="p_ctx", bufs=1, space="PSUM") as p_ctx, \
                 tc.tile_pool(name="p_kv", bufs=2, space="PSUM") as p_kv:
                self.p_kv, self.kep, self.knp = p_kv, kep, knp
                self.vap, self.smallp = vap, smallp
                pre_y = {}
                for i in range(NIN):
                    tiles = []
                    for c in range(FT):
                        y = ysp.tile([128, CHUNK], F32R, tag="ys", name="ys")
                        nc.sync.dma_start(
                            out=y, in_=I["ysT"][i, 128 * c : 128 * (c + 1),
                                                0:CHUNK].bitcast(F32R))
                        tiles.append(y)
                    pre_y[i] = tiles
                WK = [self.load_w512(I["wk"][i], w_kv, f"wk{i}")
                      for i in range(NIN)]
                WV = [self.load_w512(I["wv"][i], w_kv, f"wv{i}")
                      for i in range(NIN)]
                X = [[self.resid.tile([128, CHUNK], F32R, tag="resid",
                                      name="resid")
                      for _ in range(FT)] for _ in range(NCH)]
                for ch in range(NCH):
                    for c in range(FT):
                        nc.sync.dma_start(
                            out=X[ch][c],
                            in_=I["xT"][128 * c : 128 * (c + 1),
                                        CHUNK * ch : CHUNK * (ch + 1)
                                        ].bitcast(F32R))
                CTX = [p_ctx.tile([128, FT * HB], F32, tag=f"ctx{i}",
                                  name=f"ctx{i}") for i in range(NIN)]
                for ch in range(NCH):
                    for i in range(NIN):
                        if ch == 0:
                            yt = pre_y[i]
                        else:
                            yt = []
                            for c in range(FT):
                                y = ysp.tile([128, CHUNK], F32R, tag="ys",
                                             name="ys")
                                nc.sync.dma_start(
                                    out=y,
                                    in_=I["ysT"][i, 128 * c : 128 * (c + 1),
                                                 CHUNK * ch : CHUNK * (ch + 1)
                                                 ].bitcast(F32R))
                                yt.append(y)
                        a_ps, bo = self.ln_front(yt)
                        zy = self.ln_apply(yt, a_ps, zyp, "zy")
                        self.kv_ctx(zy, bo, WK[i], WV[i], KVSD[i][0],
                                    KVSD[i][1], CTX[i],
                                    first=(ch == 0), last=(ch == NCH - 1))
                for i in range(NIN):
                    cs = ctxsb.tile([128, FT * HB], BF16, tag="ctxsb",
                                    name="ctxsb")
                    nc.scalar.activation(out=cs, in_=CTX[i], func=AF.Copy,
                                         bias=0.0, scale=1.0)
                    nc.sync.dma_start(out=cc_in[i], in_=cs)
            nc.gpsimd.collective_compute(
                "AllReduce", OP.add, replica_groups=GROUPS,
                ins=[cc_in[:].opt()], outs=[cc_out[:].opt()])

            # ============ phase A2: q/E (overlaps AllReduce) ============
            E = self.q_exp(X, "wq", "fmsq", BQ, None, self.epool,
                           sq_engine="scalar")

            # ============ phase B: CA back + FFN1 ============
            X1 = self.attn_back(X, E, cc_out, NIN, I["wo"], BO, "ca",
                                self.resid)
            X2 = self.ffn(X1, "f1w1", "f1w2", F1B1, F1B2)

            # ============ phase C: SA kv + ctx ============
            cc2_in = dram.tile([128, FT * HB], BF16, tag="cc_sa_in",
                               name="cc_sa_in")
            cc2_out = dram.tile([128, FT * HB], BF16, tag="cc_sa_out",
                                name="cc_sa_out")
            lnpack4 = []
            with tc.tile_pool(name="w_kv2", bufs=1) as w_kv2, \
                 tc.tile_pool(name="kep2", bufs=2) as kep2, \
                 tc.tile_pool(name="knp2", bufs=2) as knp2, \
                 tc.tile_pool(name="vap2", bufs=2) as vap2, \
                 tc.tile_pool(name="smallp2", bufs=4) as smallp2, \
                 tc.tile_pool(name="ctxsb2", bufs=1) as ctxsb2, \
                 tc.tile_pool(name="p_ctx2", bufs=1, space="PSUM") as p_ctx2, \
                 tc.tile_pool(name="p_kv2", bufs=3, space="PSUM") as p_kv2:
                self.p_kv, self.kep, self.knp = p_kv2, kep2, knp2
                self.vap, self.smallp = vap2, smallp2
                SWK = self.load_w512(I["sak"], w_kv2, "sak")
                SWV = self.load_w512(I["sav"], w_kv2, "sav")
                CTX2 = p_ctx2.tile([128, FT * HB], F32, tag="ctx2", name="ctx2")
                for ch in range(NCH):
                    a_ps, bo = self.ln_front(X2[ch], sq_engine="scalar")
                    xn4 = self.ln_apply(X2[ch], a_ps, self.xn4p, "xn4")
                    lnpack4.append((bo, xn4))
                for ch in range(NCH):
                    self.kv_ctx(lnpack4[ch][1], lnpack4[ch][0], SWK, SWV,
                                SASDK, SASDV, CTX2,
                                first=(ch == 0), last=(ch == NCH - 1))
                cs2 = ctxsb2.tile([128, FT * HB], BF16, tag="ctxsb2",
                                  name="ctxsb2")
                nc.scalar.activation(out=cs2, in_=CTX2, func=AF.Copy,
                                     bias=0.0, scale=1.0)
                nc.sync.dma_start(out=cc2_in, in_=cs2)
            nc.gpsimd.collective_compute(
                "AllReduce", OP.add, replica_groups=GROUPS,
                ins=[cc2_in[:].opt()], outs=[cc2_out[:].opt()])

            # ============ phase C2: E2 (overlaps AllReduce) ============
            E2 = self.q_exp(X2, "saq", "fmssaq", BSAQ, lnpack4, self.epool)

            # ============ phase D: SA back + FFN2 ============
            X3 = self.attn_back(X2, E2, cc2_out, 1, I["sao"], BSAO, "sa",
                                self.resid)
            XF = self.ffn(X3, "f2w1", "f2w2", F2B1, F2B2)

            for ch in range(NCH):
                for m in range(FT):
                    nc.sync.dma_start(
                        out=self.out_t[128 * m : 128 * (m + 1),
                                       CHUNK * ch : CHUNK * (ch + 1)],
                        in_=XF[ch][m].bitcast(F32))


# ---------------------------------------------------------------------------
# host side
# ---------------------------------------------------------------------------
_PROGRAM = None
LAST_RESULTS = None


def _bf16(a):
    import ml_dtypes
    return np.ascontiguousarray(np.asarray(a, np.float32)).astype(
        ml_dtypes.bfloat16)


def _cols(v, nt):
    return np.ascontiguousarray(np.asarray(v, np.float32).reshape(nt, 128).T)


def _host_consts():
    sgbase = np.zeros((FT, 128, 24), np.float32)
    sel8 = np.zeros((FT, 8, 128), np.float32)
    for c in range(FT):
        for p in range(128):
            h = 2 * c + (1 if p >= 64 else 0)
            sgbase[c, p, h] = 1.0
            sel8[c, h, p] = 1.0
    return {
        "ones_c": np.ones((1, 128), np.float32),
        "ones_r": np.ones((128, 1), np.float32),
        "sgbase": sgbase,
        "sel8": sel8,  # converted below
    }


def _make_in_maps(inputs):
    f = lambda k: np.asarray(inputs[k], np.float32)

    def fold(w, g):
        return w * g[None, :]

    def fm_seed(wg):
        u = wg.sum(1)  # [out]
        return _bf16(u.reshape(FT, 1, 128))

    def tm_seed(wg, w, lb, b):
        u = wg.sum(1)
        bp = w @ lb + b
        return np.stack([u, bp])  # [2, C]

    wq_g = fold(f("ca_wq"), f("ln1_g"))
    saq_g = fold(f("sa_wq"), f("ln4_g"))
    sak_g = fold(f("sa_wk"), f("ln4_g"))
    sav_g = fold(f("sa_wv"), f("ln4_g"))
    wk_g = np.stack([fold(f("ca_wk")[i], f("ln2_g")[i]) for i in range(NIN)])
    wv_g = np.stack([fold(f("ca_wv")[i], f("ln2_g")[i]) for i in range(NIN)])
    f1w1_g = fold(f("ffn1_w1"), f("ln3_g"))
    f2w1_g = fold(f("ffn2_w1"), f("ln5_g"))

    kvsd = np.zeros((NIN, 2, 2, C), np.float32)
    for i in range(NIN):
        kvsd[i, 0] = tm_seed(wk_g[i], f("ca_wk")[i], f("ln2_b")[i],
                             f("ca_bk")[i])
        kvsd[i, 1] = tm_seed(wv_g[i], f("ca_wv")[i], f("ln2_b")[i],
                             f("ca_bv")[i])
    sasd = np.zeros((2, 2, C), np.float32)
    sasd[0] = tm_seed(sak_g, f("sa_wk"), f("ln4_b"), f("sa_bk"))
    sasd[1] = tm_seed(sav_g, f("sa_wv"), f("ln4_b"), f("sa_bv"))

    shared = {
        "wq": _bf16(wq_g.T),
        "wo": _bf16(f("ca_wo").T),
        "saq": _bf16(saq_g.T),
        "sao": _bf16(f("sa_wo").T),
        "sak": _bf16(sak_g.T),
        "sav": _bf16(sav_g.T),
        "wk": _bf16(wk_g.transpose(0, 2, 1)),
        "wv": _bf16(wv_g.transpose(0, 2, 1)),
        "f1w1": _bf16(f1w1_g.T),
        "f1w2": _bf16(f("ffn1_w2").T),
        "f2w1": _bf16(f2w1_g.T),
        "f2w2": _bf16(f("ffn2_w2").T),
        "fmsq": fm_seed(wq_g),
        "fmssaq": fm_seed(saq_g),
        "kvsd": _bf16(kvsd),
        "sasd": _bf16(sasd),
        "bq_c": _cols(f("ca_wq") @ f("ln1_b") + f("ca_bq"), FT),
        "bsaq_c": _cols(f("sa_wq") @ f("ln4_b") + f("sa_bq"), FT),
        "bo_c": _cols(f("ca_bo"), FT),
        "bsao_c": _cols(f("sa_bo"), FT),
        "f1b1_c": _cols(f("ffn1_w1") @ f("ln3_b") + f("ffn1_b1"), IT),
        "f2b1_c": _cols(f("ffn2_w1") @ f("ln5_b") + f("ffn2_b1"), IT),
        "f1b2_c": _cols(f("ffn1_b2"), FT),
        "f2b2_c": _cols(f("ffn2_b2"), FT),
    }
    hc = _host_consts()
    hc["sel8"] = _bf16(hc["sel8"])
    hc["sgbase"] = _bf16(hc["sgbase"])
    shared.update(hc)

    x = f("x")
    ys = f("ys")
    in_maps = []
    for core in range(N_CORES):
        b, half = core // 2, core % 2
        lo, hi = half * NTOK, (half + 1) * NTOK
        m = dict(shared)
        m["xT"] = np.ascontiguousarray(x[b, lo:hi, :].T)
        m["ysT"] = np.ascontiguousarray(ys[:, b, lo:hi, :].transpose(0, 2, 1))
        in_maps.append(m)
    return in_maps


def kernel(**inputs):
    global _PROGRAM, LAST_RESULTS
    if _PROGRAM is None:
        _PROGRAM = _build_program()
    nc = _PROGRAM
    in_maps = _make_in_maps(inputs)

    trace = os.environ.get("BASS_TRACE", "") not in ("", "0")
    res = run_bass_kernel_spmd(nc, in_maps, core_ids=list(range(N_CORES)),
                               trace=trace)
    LAST_RESULTS = res

    out = np.empty((B, T, C), np.float32)
    for core in range(N_CORES):
        b, half = core // 2, core % 2
        lo, hi = half * NTOK, (half + 1) * NTOK
        out[b, lo:hi, :] = res.results[core]["outT"].T
    return out
